# revision 1
# baseline (speedup 1.0000x reference)
"""Trainium2 Bass kernel for nn_DeltaAI_34703335752317 (gnn_message_passing).

Computation (see reference):
    x = relu(LN(V @ W1 + b1))   # [N, H], LN over H with eps=1e-5
    x = relu(LN(x @ W2 + b2))
    x = relu(LN(x @ W3 + b3))
    out[n] = dot(x[n], Wp[ilist[n], :, 0]) + bp[ilist[n]]
    out = where(sum|V[n]| == 0, marginals[ilist[n]], out) / temp

Strategy: pure data parallel over N across 8 cores.  Host pre-transposes V
(per-core packed [T, 128, VDIM] tiles so the contraction dim lands on SBUF
partitions with fully contiguous DMAs), folds the LN mean-centering into the
weights (z - mean(z) == V @ (W @ C) + b @ C with C = I - 1/H), and
pre-gathers the per-row output head Wp[ilist]/bp[ilist].  The device kernel
streams V^T tiles at HBM rate and runs matmuls + LN + head on chip.
"""

import numpy as np

import concourse.bacc as bacc
import concourse.bass as bass
import concourse.tile as tile
from concourse import mybir
from concourse.bass import ts
from concourse.bass_utils import run_bass_kernel_spmd

NCORES = 8
N = 65536
VDIM = 2048
HDIM = 64
LN_EPS = 1e-5

NPC = N // NCORES          # rows per core = 8192
P = 128                    # partitions
TPC = NPC // P             # row-tiles per core = 64
GRP = 8                    # row-tiles per group (8*64 = 512 psum floats = 1 bank)
NG = TPC // GRP            # groups per core = 8
KC = VDIM // P             # contraction chunks = 16

F32 = mybir.dt.float32


def _build_nc(has_b, has_g, has_be, tpc=TPC, ng=NG):
    """Build + compile the per-core Bass program (same NEFF on all cores)."""
    TPC, NG = tpc, ng  # noqa: N806 — allow small-scale builds for simulation
    nc = bacc.Bacc(
        "TRN2", target_bir_lowering=False, debug=False, num_devices=NCORES
    )

    NRG = TPC // 4  # 512-row groups per core
    RG = 512        # rows per matmul moving operand (fp32 max free dim)
    vt = nc.dram_tensor("vt", [NRG, P, KC * RG], F32, kind="ExternalInput")
    w1 = nc.dram_tensor("w1", [VDIM, HDIM], F32, kind="ExternalInput")
    w2 = nc.dram_tensor("w2", [HDIM, HDIM], F32, kind="ExternalInput")
    w3 = nc.dram_tensor("w3", [HDIM, HDIM], F32, kind="ExternalInput")
    wg = nc.dram_tensor("wg", [NG, P, GRP, HDIM], F32, kind="ExternalInput")
    bg = nc.dram_tensor("bg", [NG, P, GRP], F32, kind="ExternalInput")
    ident = nc.dram_tensor("ident", [P, P], F32, kind="ExternalInput")
    b_in = g_in = be_in = None
    if has_b:
        b_in = nc.dram_tensor("bvec", [3, P, HDIM], F32, kind="ExternalInput")
    if has_g:
        g_in = nc.dram_tensor("gvec", [3, P, HDIM], F32, kind="ExternalInput")
    if has_be:
        be_in = nc.dram_tensor("bevec", [3, P, HDIM], F32, kind="ExternalInput")
    o = nc.dram_tensor("o", [NG, P, GRP], F32, kind="ExternalOutput")

    with tile.TileContext(nc) as tc:
        with (
            tc.tile_pool(name="consts", bufs=1) as consts,
            tc.tile_pool(name="vpool", bufs=4) as vpool,
            tc.tile_pool(name="xpool", bufs=6) as xpool,
            tc.tile_pool(name="upool", bufs=4) as upool,
            tc.tile_pool(name="sqpool", bufs=3) as sqpool,
            tc.tile_pool(name="xtpool", bufs=4) as xtpool,
            tc.tile_pool(name="wgpool", bufs=2) as wgpool,
            tc.tile_pool(name="stat", bufs=6) as stat,
            tc.tile_pool(name="respool", bufs=3) as respool,
            tc.tile_pool(name="psz", bufs=2, space="PSUM") as psz,
            tc.tile_pool(name="ptr", bufs=2, space="PSUM") as ptr,
        ):
            # --- constants ---
            w1_sb = consts.tile([P, KC, HDIM], F32)
            nc.sync.dma_start(
                out=w1_sb[:], in_=w1[:].rearrange("(k p) h -> p k h", p=P)
            )
            w2_sb = consts.tile([HDIM, HDIM], F32)
            nc.sync.dma_start(out=w2_sb[:], in_=w2[:])
            w3_sb = consts.tile([HDIM, HDIM], F32)
            nc.sync.dma_start(out=w3_sb[:], in_=w3[:])
            id_sb = consts.tile([P, P], F32)
            nc.sync.dma_start(out=id_sb[:], in_=ident[:])
            eps_sb = consts.tile([P, 1], F32)
            nc.vector.memset(eps_sb[:], LN_EPS)
            b_sb = g_sb = be_sb = None
            if b_in is not None:
                b_sb = consts.tile([P, 3, HDIM], F32)
                nc.sync.dma_start(
                    out=b_sb[:], in_=b_in[:].rearrange("l p h -> p l h")
                )
            if g_in is not None:
                g_sb = consts.tile([P, 3, HDIM], F32)
                nc.sync.dma_start(
                    out=g_sb[:], in_=g_in[:].rearrange("l p h -> p l h")
                )
            if be_in is not None:
                be_sb = consts.tile([P, 3, HDIM], F32)
                nc.sync.dma_start(
                    out=be_sb[:], in_=be_in[:].rearrange("l p h -> p l h")
                )

            def ln_relu(pz, li):
                """LN (mean pre-folded into W) + relu: PSUM [P,GRP,H] -> SBUF."""
                w = pz
                if b_sb is not None:
                    wsb = upool.tile([P, GRP, HDIM], F32, tag="wsb")
                    nc.vector.tensor_add(
                        wsb[:],
                        pz[:],
                        b_sb[:, li, None, :].to_broadcast((P, GRP, HDIM)),
                    )
                    w = wsb
                sq = sqpool.tile([P, GRP, HDIM], F32)
                nc.scalar.square(sq[:], w[:])
                var = stat.tile([P, GRP], F32)
                nc.vector.reduce_sum(var[:], sq[:], axis=mybir.AxisListType.X)
                # std = sqrt(var/H + eps); inv = 1/std
                inv = stat.tile([P, GRP], F32)
                nc.scalar.activation(
                    inv[:],
                    var[:],
                    mybir.ActivationFunctionType.Sqrt,
                    bias=eps_sb[:],
                    scale=1.0 / HDIM,
                )
                nc.vector.reciprocal(inv[:], inv[:])
                u = upool.tile([P, GRP, HDIM], F32)
                nc.vector.tensor_mul(
                    u[:], w[:], inv[:, :, None].to_broadcast((P, GRP, HDIM))
                )
                if g_sb is not None:
                    nc.vector.tensor_mul(
                        u[:],
                        u[:],
                        g_sb[:, li, None, :].to_broadcast((P, GRP, HDIM)),
                    )
                if be_sb is not None:
                    nc.vector.tensor_add(
                        u[:],
                        u[:],
                        be_sb[:, li, None, :].to_broadcast((P, GRP, HDIM)),
                    )
                x = xpool.tile([P, GRP, HDIM], F32)
                nc.vector.tensor_scalar_max(x[:], u[:], 0.0)
                return x

            for g in range(NG):
                vhs = []
                for half in range(2):
                    vh = vpool.tile([P, KC, RG], F32, tag="v")
                    nc.sync.dma_start(out=vh[:], in_=vt[2 * g + half])
                    vhs.append(vh)
                wg_sb = wgpool.tile([P, GRP, HDIM], F32)
                nc.sync.dma_start(out=wg_sb[:], in_=wg[g])
                bg_sb = respool.tile([P, GRP], F32, tag="bg")
                nc.sync.dma_start(out=bg_sb[:], in_=bg[g])

                # ---- layer 1: z^T = W1C^T @ V^T (W stationary, V^T moving) ----
                pzT = ptr.tile([HDIM, 2, RG], F32, tag="pt")
                for half in range(2):
                    for k in range(KC):
                        nc.tensor.matmul(
                            pzT[:, half, :],
                            lhsT=w1_sb[:, k, :],
                            rhs=vhs[half][:, k, :],
                            start=(k == 0),
                            stop=(k == KC - 1),
                        )
                z1T = xtpool.tile([HDIM, 2, RG], F32, tag="xt")
                nc.scalar.copy(z1T[:], pzT[:])
                # transpose z^T back to rows-on-partitions [P, GRP, H]
                pz = psz.tile([P, GRP, HDIM], F32, tag="pz")
                for t in range(GRP):
                    nc.tensor.transpose(
                        pz[:, t, :],
                        z1T[:, t // 4, ts(t % 4, P)],
                        id_sb[:HDIM, :HDIM],
                    )
                x = ln_relu(pz, 0)

                # ---- layers 2,3: transpose x, then z = x @ W ----
                for li, w_sb in ((1, w2_sb), (2, w3_sb)):
                    pt = ptr.tile([HDIM, GRP, P], F32, tag="pt")
                    for t in range(GRP):
                        nc.tensor.transpose(pt[:, t, :], x[:, t, :], id_sb[:])
                    xt = xtpool.tile([HDIM, GRP, P], F32)
                    nc.scalar.copy(xt[:], pt[:])
                    pz2 = psz.tile([P, GRP, HDIM], F32, tag="pz")
                    for t in range(GRP):
                        nc.tensor.matmul(
                            pz2[:, t, :],
                            lhsT=xt[:, t, :],
                            rhs=w_sb[:],
                            start=True,
                            stop=True,
                        )
                    x = ln_relu(pz2, li)

                # ---- head: out = dot(x, wg) + bg ----
                scr = sqpool.tile([P, GRP, HDIM], F32, tag="scr")
                nc.vector.tensor_mul(scr[:], x[:], wg_sb[:])
                dot = stat.tile([P, GRP], F32, tag="dot")
                nc.vector.reduce_sum(dot[:], scr[:], axis=mybir.AxisListType.X)
                res = respool.tile([P, GRP], F32, tag="res")
                nc.vector.tensor_add(res[:], dot[:], bg_sb[:])
                nc.sync.dma_start(out=o[g], in_=res[:])

    nc.compile()
    return nc


_NC_CACHE = {}
LAST_RESULTS = None


def _get_nc(has_b, has_g, has_be):
    key = (has_b, has_g, has_be)
    if key not in _NC_CACHE:
        _NC_CACHE[key] = _build_nc(has_b, has_g, has_be)
    return _NC_CACHE[key]


def _center(w):
    # w @ (I - 1/H): subtract row-means, in float64 for exactness
    w64 = np.asarray(w, np.float64)
    return (w64 - w64.mean(axis=-1, keepdims=True)).astype(np.float32)


def kernel(
    V, ilist, temp, W1, b1, g1, be1, W2, b2, g2, be2, W3, b3, g3, be3,
    Wp, bp, marginals,
):
    V = np.asarray(V, np.float32)
    ilist_np = np.asarray(ilist)
    W1c = _center(np.asarray(W1))
    W2c = _center(np.asarray(W2))
    W3c = _center(np.asarray(W3))
    bs = [np.asarray(b, np.float64) for b in (b1, b2, b3)]
    bs = np.stack([(b - b.mean()).astype(np.float32) for b in bs])  # [3, H]
    gs = np.stack([np.asarray(g, np.float32) for g in (g1, g2, g3)])
    bes = np.stack([np.asarray(b, np.float32) for b in (be1, be2, be3)])

    has_b = bool(np.any(bs))
    has_g = not bool(np.all(gs == 1.0))
    has_be = bool(np.any(bes))
    nc = _get_nc(has_b, has_g, has_be)

    # pre-gathered per-row output head
    Wg = np.ascontiguousarray(Wp[ilist_np, :, 0]).astype(np.float32)  # [N, H]
    bgv = np.ascontiguousarray(bp[ilist_np, 0, 0]).astype(np.float32)  # [N]

    shared = {
        "w1": W1c,
        "w2": W2c,
        "w3": W3c,
        "ident": np.eye(P, dtype=np.float32),
    }
    if has_b:
        shared["bvec"] = np.ascontiguousarray(
            np.broadcast_to(bs[:, None, :], (3, P, HDIM))
        )
    if has_g:
        shared["gvec"] = np.ascontiguousarray(
            np.broadcast_to(gs[:, None, :], (3, P, HDIM))
        )
    if has_be:
        shared["bevec"] = np.ascontiguousarray(
            np.broadcast_to(bes[:, None, :], (3, P, HDIM))
        )

    in_maps = []
    for c in range(NCORES):
        sl = slice(c * NPC, (c + 1) * NPC)
        # packed V^T row-groups: vt[rg, p, k*512 + r] = V[c*NPC + rg*512 + r, k*128 + p]
        vc = np.ascontiguousarray(
            V[sl].reshape(TPC // 4, 512, KC, P).transpose(0, 3, 2, 1)
        ).reshape(TPC // 4, P, KC * 512)
        wgc = np.ascontiguousarray(
            Wg[sl].reshape(NG, GRP, P, HDIM).transpose(0, 2, 1, 3)
        )
        bgc = np.ascontiguousarray(
            bgv[sl].reshape(NG, GRP, P).transpose(0, 2, 1)
        )
        in_maps.append({"vt": vc, "wg": wgc, "bg": bgc, **shared})

    kres = run_bass_kernel_spmd(nc, in_maps, core_ids=list(range(NCORES)))
    global LAST_RESULTS
    LAST_RESULTS = kres
    out = np.empty(N, np.float32)
    for c in range(NCORES):
        oc = kres.results[c]["o"]  # [NG, P, GRP]
        out[c * NPC : (c + 1) * NPC] = oc.transpose(0, 2, 1).reshape(NPC)

    # epilogue on host: zero-row marginals + temperature
    zero_rows = np.abs(V).sum(axis=1) == 0.0
    if zero_rows.any():
        out = np.where(
            zero_rows, np.asarray(marginals, np.float32)[ilist_np], out
        )
    t = np.float32(np.asarray(temp))
    if t != 1.0:
        out = (out / t).astype(np.float32)
    return out



# revision 2
# speedup vs baseline: 2.0875x; 2.0875x over previous
"""Trainium2 Bass kernel for nn_DeltaAI_34703335752317 (gnn_message_passing).

Computation (see reference):
    x = relu(LN(V @ W1 + b1))   # [N, H], LN over H with eps=1e-5
    x = relu(LN(x @ W2 + b2))
    x = relu(LN(x @ W3 + b3))
    out[n] = dot(x[n], Wp[ilist[n], :, 0]) + bp[ilist[n]]
    out = where(sum|V[n]| == 0, marginals[ilist[n]], out) / temp

Strategy: pure data parallel over N across 8 cores.  Host pre-transposes V
(per-core packed [T, 128, VDIM] tiles so the contraction dim lands on SBUF
partitions with fully contiguous DMAs), folds the LN mean-centering into the
weights (z - mean(z) == V @ (W @ C) + b @ C with C = I - 1/H), and
pre-gathers the per-row output head Wp[ilist]/bp[ilist].  The device kernel
streams V^T tiles at HBM rate and runs matmuls + LN + head on chip.

All streamed data is fp16: halves HBM traffic vs fp32 and runs the PE at
1 cycle/row instead of fp32's 4 (fp32 matmuls issue as 2 half-speed passes).
PSUM accumulation and LN statistics stay fp32.  Verified numerically on the
host: fp16-chain max rel err ~1e-3 vs the 2e-2 gate (fp8 V was measured at
2.3e-2 — over the gate — hence fp16).
"""

import numpy as np

import concourse.bacc as bacc
import concourse.bass as bass
import concourse.tile as tile
from concourse import mybir
from concourse.bass import ts
from concourse.bass_utils import run_bass_kernel_spmd

NCORES = 8
N = 65536
VDIM = 2048
HDIM = 64
LN_EPS = 1e-5

NPC = N // NCORES          # rows per core = 8192
P = 128                    # partitions
TPC = NPC // P             # row-tiles per core = 64
GRP = 8                    # row-tiles per group (8*64 = 512 psum floats = 1 bank)
NG = TPC // GRP            # groups per core = 8
KC = VDIM // P             # contraction chunks = 16

F32 = mybir.dt.float32
F16 = mybir.dt.float16


def _build_nc(has_b, has_g, has_be, tpc=TPC, ng=NG):
    """Build + compile the per-core Bass program (same NEFF on all cores)."""
    TPC, NG = tpc, ng  # noqa: N806 — allow small-scale builds for simulation
    nc = bacc.Bacc(
        "TRN2", target_bir_lowering=False, debug=False, num_devices=NCORES
    )

    NRG = TPC // 4  # 512-row groups per core
    RG = 512        # rows per matmul moving operand (fp32 max free dim)
    vt = nc.dram_tensor("vt", [NRG, P, KC * RG], F16, kind="ExternalInput")
    w1 = nc.dram_tensor("w1", [VDIM, HDIM], F16, kind="ExternalInput")
    w2 = nc.dram_tensor("w2", [HDIM, HDIM], F16, kind="ExternalInput")
    w3 = nc.dram_tensor("w3", [HDIM, HDIM], F16, kind="ExternalInput")
    wg = nc.dram_tensor("wg", [NG, P, GRP, HDIM], F16, kind="ExternalInput")
    bg = nc.dram_tensor("bg", [NG, P, GRP], F32, kind="ExternalInput")
    ident = nc.dram_tensor("ident", [P, P], F16, kind="ExternalInput")
    b_in = g_in = be_in = None
    if has_b:
        b_in = nc.dram_tensor("bvec", [3, P, HDIM], F32, kind="ExternalInput")
    if has_g:
        g_in = nc.dram_tensor("gvec", [3, P, HDIM], F32, kind="ExternalInput")
    if has_be:
        be_in = nc.dram_tensor("bevec", [3, P, HDIM], F32, kind="ExternalInput")
    o = nc.dram_tensor("o", [NG, P, GRP], F32, kind="ExternalOutput")

    with tile.TileContext(nc) as tc:
        with (
            tc.tile_pool(name="consts", bufs=1) as consts,
            tc.tile_pool(name="vpool", bufs=4) as vpool,
            tc.tile_pool(name="xpool", bufs=6) as xpool,
            tc.tile_pool(name="upool", bufs=4) as upool,
            tc.tile_pool(name="sqpool", bufs=3) as sqpool,
            tc.tile_pool(name="xtpool", bufs=4) as xtpool,
            tc.tile_pool(name="wgpool", bufs=2) as wgpool,
            tc.tile_pool(name="stat", bufs=6) as stat,
            tc.tile_pool(name="respool", bufs=3) as respool,
            tc.tile_pool(name="psz", bufs=2, space="PSUM") as psz,
            tc.tile_pool(name="ptr", bufs=2, space="PSUM") as ptr,
        ):
            # --- constants ---
            w1_sb = consts.tile([P, KC, HDIM], F16)
            nc.sync.dma_start(
                out=w1_sb[:], in_=w1[:].rearrange("(k p) h -> p k h", p=P)
            )
            w2_sb = consts.tile([HDIM, HDIM], F16)
            nc.sync.dma_start(out=w2_sb[:], in_=w2[:])
            w3_sb = consts.tile([HDIM, HDIM], F16)
            nc.sync.dma_start(out=w3_sb[:], in_=w3[:])
            id_sb = consts.tile([P, P], F16)
            nc.sync.dma_start(out=id_sb[:], in_=ident[:])
            eps_sb = consts.tile([P, 1], F32)
            nc.vector.memset(eps_sb[:], LN_EPS)
            b_sb = g_sb = be_sb = None
            if b_in is not None:
                b_sb = consts.tile([P, 3, HDIM], F32)
                nc.sync.dma_start(
                    out=b_sb[:], in_=b_in[:].rearrange("l p h -> p l h")
                )
            if g_in is not None:
                g_sb = consts.tile([P, 3, HDIM], F32)
                nc.sync.dma_start(
                    out=g_sb[:], in_=g_in[:].rearrange("l p h -> p l h")
                )
            if be_in is not None:
                be_sb = consts.tile([P, 3, HDIM], F32)
                nc.sync.dma_start(
                    out=be_sb[:], in_=be_in[:].rearrange("l p h -> p l h")
                )

            def ln_relu(pz, li):
                """LN (mean pre-folded into W) + relu: PSUM [P,GRP,H] -> SBUF."""
                w = pz
                if b_sb is not None:
                    wsb = upool.tile([P, GRP, HDIM], F32, tag="wsb")
                    nc.vector.tensor_add(
                        wsb[:],
                        pz[:],
                        b_sb[:, li, None, :].to_broadcast((P, GRP, HDIM)),
                    )
                    w = wsb
                sq = sqpool.tile([P, GRP, HDIM], F32)
                nc.scalar.square(sq[:], w[:])
                var = stat.tile([P, GRP], F32)
                nc.vector.reduce_sum(var[:], sq[:], axis=mybir.AxisListType.X)
                # std = sqrt(var/H + eps); inv = 1/std
                inv = stat.tile([P, GRP], F32)
                nc.scalar.activation(
                    inv[:],
                    var[:],
                    mybir.ActivationFunctionType.Sqrt,
                    bias=eps_sb[:],
                    scale=1.0 / HDIM,
                )
                nc.vector.reciprocal(inv[:], inv[:])
                u = upool.tile([P, GRP, HDIM], F32)
                nc.vector.tensor_mul(
                    u[:], w[:], inv[:, :, None].to_broadcast((P, GRP, HDIM))
                )
                if g_sb is not None:
                    nc.vector.tensor_mul(
                        u[:],
                        u[:],
                        g_sb[:, li, None, :].to_broadcast((P, GRP, HDIM)),
                    )
                if be_sb is not None:
                    nc.vector.tensor_add(
                        u[:],
                        u[:],
                        be_sb[:, li, None, :].to_broadcast((P, GRP, HDIM)),
                    )
                x = xpool.tile([P, GRP, HDIM], F16)
                nc.vector.tensor_scalar_max(x[:], u[:], 0.0)
                return x

            for g in range(NG):
                vhs = []
                for half in range(2):
                    vh = vpool.tile([P, KC, RG], F16, tag="v")
                    nc.sync.dma_start(out=vh[:], in_=vt[2 * g + half])
                    vhs.append(vh)
                wg_sb = wgpool.tile([P, GRP, HDIM], F16)
                nc.sync.dma_start(out=wg_sb[:], in_=wg[g])
                bg_sb = respool.tile([P, GRP], F32, tag="bg")
                nc.sync.dma_start(out=bg_sb[:], in_=bg[g])

                # ---- layer 1: z^T = W1C^T @ V^T (W stationary, V^T moving) ----
                pzT = ptr.tile([HDIM, 2, RG], F32, tag="pt")
                for half in range(2):
                    for k in range(KC):
                        nc.tensor.matmul(
                            pzT[:, half, :],
                            lhsT=w1_sb[:, k, :],
                            rhs=vhs[half][:, k, :],
                            start=(k == 0),
                            stop=(k == KC - 1),
                        )
                z1T = xtpool.tile([HDIM, 2, RG], F16, tag="xt")
                nc.scalar.copy(z1T[:], pzT[:])
                # transpose z^T back to rows-on-partitions [P, GRP, H]
                pz = psz.tile([P, GRP, HDIM], F16, tag="pz")
                for t in range(GRP):
                    nc.tensor.transpose(
                        pz[:, t, :],
                        z1T[:, t // 4, ts(t % 4, P)],
                        id_sb[:HDIM, :HDIM],
                    )
                x = ln_relu(pz, 0)

                # ---- layers 2,3: transpose x, then z = x @ W ----
                for li, w_sb in ((1, w2_sb), (2, w3_sb)):
                    pt = ptr.tile([HDIM, GRP, P], F16, tag="pt")
                    for t in range(GRP):
                        nc.tensor.transpose(pt[:, t, :], x[:, t, :], id_sb[:])
                    xt = xtpool.tile([HDIM, GRP, P], F16)
                    nc.scalar.copy(xt[:], pt[:])
                    pz2 = psz.tile([P, GRP, HDIM], F32, tag="pz")
                    for t in range(GRP):
                        nc.tensor.matmul(
                            pz2[:, t, :],
                            lhsT=xt[:, t, :],
                            rhs=w_sb[:],
                            start=True,
                            stop=True,
                        )
                    x = ln_relu(pz2, li)

                # ---- head: out = dot(x, wg) + bg ----
                scr = sqpool.tile([P, GRP, HDIM], F32, tag="scr")
                nc.vector.tensor_mul(scr[:], x[:], wg_sb[:])
                dot = stat.tile([P, GRP], F32, tag="dot")
                nc.vector.reduce_sum(dot[:], scr[:], axis=mybir.AxisListType.X)
                res = respool.tile([P, GRP], F32, tag="res")
                nc.vector.tensor_add(res[:], dot[:], bg_sb[:])
                nc.sync.dma_start(out=o[g], in_=res[:])

    nc.compile()
    return nc


_NC_CACHE = {}
LAST_RESULTS = None


def _get_nc(has_b, has_g, has_be):
    key = (has_b, has_g, has_be)
    if key not in _NC_CACHE:
        _NC_CACHE[key] = _build_nc(has_b, has_g, has_be)
    return _NC_CACHE[key]


def _center(w):
    # w @ (I - 1/H): subtract row-means, in float64 for exactness
    w64 = np.asarray(w, np.float64)
    return (w64 - w64.mean(axis=-1, keepdims=True)).astype(np.float32)


def kernel(
    V, ilist, temp, W1, b1, g1, be1, W2, b2, g2, be2, W3, b3, g3, be3,
    Wp, bp, marginals,
):
    V = np.asarray(V, np.float32)
    ilist_np = np.asarray(ilist)
    W1c = _center(np.asarray(W1)).astype(np.float16)
    W2c = _center(np.asarray(W2)).astype(np.float16)
    W3c = _center(np.asarray(W3)).astype(np.float16)
    bs = [np.asarray(b, np.float64) for b in (b1, b2, b3)]
    bs = np.stack([(b - b.mean()).astype(np.float32) for b in bs])  # [3, H]
    gs = np.stack([np.asarray(g, np.float32) for g in (g1, g2, g3)])
    bes = np.stack([np.asarray(b, np.float32) for b in (be1, be2, be3)])

    has_b = bool(np.any(bs))
    has_g = not bool(np.all(gs == 1.0))
    has_be = bool(np.any(bes))
    nc = _get_nc(has_b, has_g, has_be)

    # pre-gathered per-row output head
    Wg = np.ascontiguousarray(Wp[ilist_np, :, 0]).astype(np.float16)  # [N, H]
    bgv = np.ascontiguousarray(bp[ilist_np, 0, 0]).astype(np.float32)  # [N]

    shared = {
        "w1": W1c,
        "w2": W2c,
        "w3": W3c,
        "ident": np.eye(P, dtype=np.float16),
    }
    if has_b:
        shared["bvec"] = np.ascontiguousarray(
            np.broadcast_to(bs[:, None, :], (3, P, HDIM))
        )
    if has_g:
        shared["gvec"] = np.ascontiguousarray(
            np.broadcast_to(gs[:, None, :], (3, P, HDIM))
        )
    if has_be:
        shared["bevec"] = np.ascontiguousarray(
            np.broadcast_to(bes[:, None, :], (3, P, HDIM))
        )

    V16 = V.astype(np.float16)
    in_maps = []
    for c in range(NCORES):
        sl = slice(c * NPC, (c + 1) * NPC)
        # packed V^T row-groups: vt[rg, p, k*512 + r] = V[c*NPC + rg*512 + r, k*128 + p]
        vc = np.ascontiguousarray(
            V16[sl].reshape(TPC // 4, 512, KC, P).transpose(0, 3, 2, 1)
        ).reshape(TPC // 4, P, KC * 512)
        wgc = np.ascontiguousarray(
            Wg[sl].reshape(NG, GRP, P, HDIM).transpose(0, 2, 1, 3)
        )
        bgc = np.ascontiguousarray(
            bgv[sl].reshape(NG, GRP, P).transpose(0, 2, 1)
        )
        in_maps.append({"vt": vc, "wg": wgc, "bg": bgc, **shared})

    kres = run_bass_kernel_spmd(nc, in_maps, core_ids=list(range(NCORES)))
    global LAST_RESULTS
    LAST_RESULTS = kres
    out = np.empty(N, np.float32)
    for c in range(NCORES):
        oc = kres.results[c]["o"]  # [NG, P, GRP]
        out[c * NPC : (c + 1) * NPC] = oc.transpose(0, 2, 1).reshape(NPC)

    # epilogue on host: zero-row marginals + temperature
    zero_rows = np.abs(V).sum(axis=1) == 0.0
    if zero_rows.any():
        out = np.where(
            zero_rows, np.asarray(marginals, np.float32)[ilist_np], out
        )
    t = np.float32(np.asarray(temp))
    if t != 1.0:
        out = (out / t).astype(np.float32)
    return out
